# revision 4
# baseline (speedup 1.0000x reference)
"""DARNN (dual-stage attention RNN) Trainium2 kernel.

Data-parallel over batch: 8 NeuronCores, 256 batch rows each, weights
replicated. Full inputs in, full output out.

Layout strategy (per core, B=256 as 2 chunks of 128 partitions):
  - Recurrent states kept transposed: hT/cT/dT/dsT are [H=128p, B=256].
  - Encoder input attention:  e[b,f] = sum_k W2[k]*tanh(PX[b,f,k]+phc[b,k])
    with PX = X-dependent part precomputed once via PE;  phc per step via PE;
    broadcast-add + tanh + weighted tree-reduce on DVE/ACT in [b,f,k] layout
    (softmax over f is then a free-dim reduction).
  - Decoder temporal attention mirrors it in [b,w,n] layout with
    PH = Hs-dependent part precomputed once.
  - sigmoid(x) computed as 0.5*tanh(0.5 x)+0.5 so the whole kernel needs a
    single ACT table set (exp_and_others: exp + tanh).
  - Large resident tensors (PX, HsT, Hs2, PH, attention scratch) in bf16;
    all matmul accumulation and softmax/LSTM state math in fp32.
"""

import os
import sys

import numpy as np

sys.path.insert(0, "/opt/trn_rl_repo")

import concourse.bacc as bacc
import concourse.bass as bass
import concourse.mybir as mybir
import concourse.tile as tile
from concourse import masks
from concourse.bass_utils import run_bass_kernel_spmd

F32 = mybir.dt.float32
BF16 = mybir.dt.bfloat16
AF = mybir.ActivationFunctionType
ALU = mybir.AluOpType
AX = mybir.AxisListType

B, WLEN, F, H = 2048, 64, 128, 128
NCORES = 8
BL = B // NCORES          # 256 rows per core
NCH = BL // 128           # 2 partition chunks

WEIGHT_SPECS = {
    "ia_W1": (WLEN, WLEN + 2 * H), "ia_b1": (WLEN,),
    "ia_W2": (1, WLEN), "ia_b2": (1,),
    "enc_Wih": (4 * H, F), "enc_Whh": (4 * H, H),
    "enc_bih": (4 * H,), "enc_bhh": (4 * H,),
    "ta_W1": (H, 3 * H), "ta_b1": (H,),
    "ta_W2": (1, H), "ta_b2": (1,),
    "dec_Wih": (4 * H, 1), "dec_Whh": (4 * H, H),
    "dec_bih": (4 * H,), "dec_bhh": (4 * H,),
    "l1_W": (1, 1 + H), "l1_b": (1,),
    "l2_W": (H, 2 * H), "l2_b": (H,),
    "l3_W": (1, H), "l3_b": (1,),
}


def _bcast(ap, mid):
    """[P, n] -> [P, mid, n] with a stride-0 middle dim."""
    return ap.unsqueeze(1).broadcast_to([ap.shape[0], mid, ap.shape[1]])


def build_kernel(tc, out_ap, ins):
    from contextlib import ExitStack

    nc = tc.nc
    Xap = ins["X"]

    stack = ExitStack()
    with stack:
        # ------------------------------------------------------------------
        # persistent pools
        # ------------------------------------------------------------------
        wp = stack.enter_context(tc.tile_pool(name="weights", bufs=1))
        pst = stack.enter_context(tc.tile_pool(name="state", bufs=1))

        identity = wp.tile([128, 128], F32, tag="ident32")
        masks.make_identity(nc, identity)
        identity_bf = wp.tile([128, 128], BF16, tag="identbf")
        masks.make_identity(nc, identity_bf)
        ones1 = wp.tile([1, 128], F32, tag="ones1")
        nc.vector.memset(ones1, 1.0)

        def load(name, src, shape, dtype=F32):
            t = wp.tile(list(shape), dtype, tag=name)
            nc.sync.dma_start(t, src)
            return t

        iaW1 = ins["ia_W1"]
        W1hT = load("W1hT", iaW1[:, WLEN:WLEN + H].rearrange("a b -> b a"), [H, WLEN])
        W1cT = load("W1cT", iaW1[:, WLEN + H:].rearrange("a b -> b a"), [H, WLEN])
        W1xT = load("W1xT", iaW1[:, :WLEN].rearrange("a b -> b a"), [WLEN, WLEN])
        b1row = load("b1row", ins["ia_b1"].unsqueeze(0), [1, WLEN])
        W2row = load("W2row", ins["ia_W2"], [1, WLEN])
        WihT = load("WihT", ins["enc_Wih"].rearrange("a b -> b a"), [F, 4 * H])
        WhhT = load("WhhT", ins["enc_Whh"].rearrange("a b -> b a"), [H, 4 * H])
        bihT = load("bihT", ins["enc_bih"].rearrange("(g h) -> h g", g=4), [128, 4])
        bhhT = load("bhhT", ins["enc_bhh"].rearrange("(g h) -> h g", g=4), [128, 4])
        benc = wp.tile([128, 4], F32, tag="benc")
        nc.vector.tensor_add(benc, bihT, bhhT)
        bench = wp.tile([128, 4], F32, tag="bench")
        nc.vector.tensor_scalar_mul(bench, benc, 0.5)

        taW1 = ins["ta_W1"]
        taW1hT32 = load("taW1hT32", taW1[:, :H].rearrange("a b -> b a"), [H, H])
        taW1hT = wp.tile([H, H], BF16, tag="taW1hT")
        nc.vector.tensor_copy(taW1hT, taW1hT32)
        taW1dT = load("taW1dT", taW1[:, H:2 * H].rearrange("a b -> b a"), [H, H])
        taW1sT = load("taW1sT", taW1[:, 2 * H:].rearrange("a b -> b a"), [H, H])
        tab1row = load("tab1row", ins["ta_b1"].unsqueeze(0), [1, H])
        taW2row = load("taW2row", ins["ta_W2"], [1, H])
        decWihR = load("decWihR", ins["dec_Wih"].rearrange("a b -> b a"), [1, 4 * H])
        decWhhT = load("decWhhT", ins["dec_Whh"].rearrange("a b -> b a"), [H, 4 * H])
        dbihT = load("dbihT", ins["dec_bih"].rearrange("(g h) -> h g", g=4), [128, 4])
        dbhhT = load("dbhhT", ins["dec_bhh"].rearrange("(g h) -> h g", g=4), [128, 4])
        bdec = wp.tile([128, 4], F32, tag="bdec")
        nc.vector.tensor_add(bdec, dbihT, dbhhT)
        bdech = wp.tile([128, 4], F32, tag="bdech")
        nc.vector.tensor_scalar_mul(bdech, bdec, 0.5)

        l1wct = load("l1wct", ins["l1_W"][:, 1:].rearrange("a b -> b a"), [H, 1])
        l1w0 = load("l1w0", ins["l1_W"][:, 0:1], [1, 1])
        l1brow = load("l1brow", ins["l1_b"].unsqueeze(0), [1, 1])
        l2WctT = load("l2WctT", ins["l2_W"][:, :H].rearrange("a b -> b a"), [H, H])
        l2WdT = load("l2WdT", ins["l2_W"][:, H:].rearrange("a b -> b a"), [H, H])
        l2bcol = load("l2bcol", ins["l2_b"].rearrange("(a b) -> a b", b=1), [H, 1])
        l3wT = load("l3wT", ins["l3_W"].rearrange("a b -> b a"), [H, 1])
        l3brow = load("l3brow", ins["l3_b"].unsqueeze(0), [1, 1])
        l3bh = wp.tile([1, 1], F32, tag="l3bh")
        nc.vector.tensor_scalar_mul(l3bh, l3brow, 0.5)

        # replicated rows (for DVE broadcasts along the free dim)
        with tc.tile_pool(name="repps", bufs=2, space="PSUM") as repps:
            rp = repps.tile([128, WLEN], F32, tag="rep")
            nc.tensor.matmul(rp, lhsT=ones1, rhs=W2row, start=True, stop=True)
            W2rep = wp.tile([128, WLEN], BF16, tag="W2rep")
            nc.vector.tensor_copy(W2rep, rp)
            rp2 = repps.tile([128, H], F32, tag="rep")
            nc.tensor.matmul(rp2, lhsT=ones1, rhs=taW2row, start=True, stop=True)
            taW2rep = wp.tile([128, H], BF16, tag="taW2rep")
            nc.vector.tensor_copy(taW2rep, rp2)

        # dummy accumulator outputs for affine_mul_reduce
        dum = stack.enter_context(tc.tile_pool(name="dum", bufs=4))

        def amr(out, in0, in1):
            d = dum.tile([128, 1], F32, tag="dum")
            nc.vector.affine_mul_reduce(out=out, accum_out=d, in0=in0, in1=in1,
                                        scale=0.5, bias=0.5)

        # ------------------------------------------------------------------
        # encoder-lifetime tensors
        # ------------------------------------------------------------------
        px_stack = ExitStack()
        pxp = px_stack.enter_context(tc.tile_pool(name="px", bufs=1))
        PX = [pxp.tile([128, F, WLEN], BF16, tag=f"px{ch}", name=f"px{ch}")
              for ch in range(NCH)]

        hsT_stack = ExitStack()
        hsp = hsT_stack.enter_context(tc.tile_pool(name="hsT", bufs=1, side="right"))
        HsT = hsp.tile([H, WLEN * BL], BF16, tag="HsT")
        HsTv = HsT.rearrange("p (w b) -> p w b", w=WLEN)

        # ---- PX build: PX[b, f, k] = sum_j X[b, j, f] W1x[k, j] + b1[k]
        with tc.tile_pool(name="xt1", bufs=1) as xt1p, \
             tc.tile_pool(name="pxps", bufs=4, space="PSUM") as pxps:
            for ch in range(NCH):
                bs = slice(ch * 128, (ch + 1) * 128)
                xt1 = xt1p.tile([WLEN, 128, F], F32, tag="xt1")
                nc.sync.dma_start(xt1, Xap[bs, :, :].rearrange("b w f -> w b f"))
                for f in range(F):
                    ps = pxps.tile([128, WLEN], F32, tag="pxmm")
                    nc.tensor.matmul(ps, lhsT=xt1[:, :, f], rhs=W1xT,
                                     start=True, stop=False)
                    nc.tensor.matmul(ps, lhsT=ones1, rhs=b1row,
                                     start=False, stop=True)
                    nc.vector.tensor_copy(PX[ch][:, f, :], ps)

        # ------------------------------------------------------------------
        # encoder loop
        # ------------------------------------------------------------------
        enc = ExitStack()
        p_xt = enc.enter_context(tc.tile_pool(name="xt", bufs=3))
        p_up = enc.enter_context(tc.tile_pool(name="up", bufs=2))
        p_u = enc.enter_context(tc.tile_pool(name="u", bufs=2))
        p_tr = enc.enter_context(tc.tile_pool(name="tr", bufs=2))
        p_e = enc.enter_context(tc.tile_pool(name="e", bufs=4))
        p_s = enc.enter_context(tc.tile_pool(name="s", bufs=4))
        p_phcb = enc.enter_context(tc.tile_pool(name="phcb", bufs=2))
        p_teffT = enc.enter_context(tc.tile_pool(name="teffT", bufs=2))
        p_th = enc.enter_context(tc.tile_pool(name="th", bufs=6))
        p_tmp = enc.enter_context(tc.tile_pool(name="tmp", bufs=3))
        p_st = enc.enter_context(tc.tile_pool(name="st", bufs=2))
        ps_phc = enc.enter_context(tc.tile_pool(name="psphc", bufs=2, space="PSUM"))
        ps_t = enc.enter_context(tc.tile_pool(name="pst", bufs=2, space="PSUM"))
        ps_g = enc.enter_context(tc.tile_pool(name="psg", bufs=4, space="PSUM"))

        hT = pst.tile([H, BL], F32, tag="h0")
        cT = pst.tile([H, BL], F32, tag="c0")
        nc.vector.memset(hT, 0.0)
        nc.vector.memset(cT, 0.0)

        for t in range(WLEN):
            t_effT = p_teffT.tile([F, BL], F32, tag="teffT")
            for ch in range(NCH):
                bs = slice(ch * 128, (ch + 1) * 128)
                xt = p_xt.tile([128, F], F32, tag="xt")
                nc.sync.dma_start(xt, Xap[bs, t, :])
                # phc[b, k] = h W1h^T + c W1c^T
                pps = ps_phc.tile([128, WLEN], F32, tag="phc")
                nc.tensor.matmul(pps, lhsT=hT[:, bs], rhs=W1hT, start=True, stop=False)
                nc.tensor.matmul(pps, lhsT=cT[:, bs], rhs=W1cT, start=False, stop=True)
                phcb = p_phcb.tile([128, WLEN], BF16, tag="phcb")
                nc.vector.tensor_copy(phcb, pps)
                # u = tanh(PX + phc)
                up = p_up.tile([128, F, WLEN], BF16, tag="up")
                nc.vector.tensor_tensor(up, PX[ch], _bcast(phcb, F), op=ALU.add)
                uu = p_u.tile([128, F, WLEN], BF16, tag="u")
                nc.scalar.activation(uu, up, AF.Tanh)
                # e = sum_k W2[k] * u[..., k]   (mul + binary tree reduce)
                wu = p_up.tile([128, F, WLEN], BF16, tag="up")
                nc.vector.tensor_tensor(wu, uu, _bcast(W2rep, F), op=ALU.mult)
                r = wu
                for sz in (32, 16, 8, 4, 2):
                    nxt = p_tr.tile([128, F, sz], BF16, tag=f"r{sz}")
                    nc.vector.tensor_tensor(nxt, r[:, :, :sz], r[:, :, sz:2 * sz],
                                            op=ALU.add)
                    r = nxt
                e = p_e.tile([128, F], F32, tag="e")
                nc.vector.tensor_tensor(e, r[:, :, 0], r[:, :, 1], op=ALU.add)
                # softmax over f (values are bounded, skip the max-subtract)
                ex = p_e.tile([128, F], F32, tag="e")
                nc.scalar.activation(ex, e, AF.Exp)
                S = p_s.tile([128, 1], F32, tag="s")
                nc.vector.reduce_sum(S, ex, axis=AX.X)
                Sr = p_s.tile([128, 1], F32, tag="s")
                nc.vector.reciprocal(Sr, S)
                al = p_e.tile([128, F], F32, tag="e")
                nc.vector.tensor_scalar_mul(al, ex, Sr)
                # t_eff = alpha * x_t, then transpose to [f, b]
                te = p_e.tile([128, F], F32, tag="e")
                nc.vector.tensor_mul(te, al, xt)
                tps = ps_t.tile([128, 128], F32, tag="tT")
                nc.tensor.transpose(tps, te, identity)
                nc.vector.tensor_copy(t_effT[:, bs], tps)

            # LSTM gates (transposed): g_i[j, b]
            th = {}
            for i, nm in enumerate("ifgo"):
                gp = ps_g.tile([H, BL], F32, tag="g")
                nc.tensor.matmul(gp, lhsT=WihT[:, i * H:(i + 1) * H], rhs=t_effT,
                                 start=True, stop=False)
                nc.tensor.matmul(gp, lhsT=WhhT[:, i * H:(i + 1) * H], rhs=hT,
                                 start=False, stop=True)
                tht = p_th.tile([H, BL], F32, tag="th")
                if nm == "g":
                    nc.scalar.activation(tht, gp, AF.Tanh, bias=benc[:, 2:3])
                else:
                    nc.scalar.activation(tht, gp, AF.Tanh, bias=bench[:, i:i + 1],
                                         scale=0.5)
                th[nm] = tht
            # c' = sig(f)c + sig(i)tanh(g);  h' = sig(o)tanh(c')   [sig via tanh]
            hN = p_st.tile([H, BL], F32, tag="h")
            cN = p_st.tile([H, BL], F32, tag="c")
            t1 = p_tmp.tile([H, BL], F32, tag="tmp")
            amr(t1, th["f"], cT)
            t2 = p_tmp.tile([H, BL], F32, tag="tmp")
            amr(t2, th["i"], th["g"])
            nc.vector.tensor_add(cN, t1, t2)
            thc = p_th.tile([H, BL], F32, tag="th")
            nc.scalar.activation(thc, cN, AF.Tanh)
            amr(hN, th["o"], thc)
            nc.vector.tensor_copy(HsTv[:, t, :], hN)
            hT, cT = hN, cN

        enc.close()
        px_stack.close()

        # ------------------------------------------------------------------
        # decoder precompute: Hs2[b, n, w] and PH[b, w, n]
        # ------------------------------------------------------------------
        dec_stack = ExitStack()
        h2p = dec_stack.enter_context(tc.tile_pool(name="hs2", bufs=1))
        php = dec_stack.enter_context(tc.tile_pool(name="ph", bufs=1))
        Hs2 = [h2p.tile([128, H, WLEN], BF16, tag=f"hs2_{ch}", name=f"hs2_{ch}")
               for ch in range(NCH)]
        PH = [php.tile([128, WLEN, H], BF16, tag=f"ph{ch}", name=f"ph{ch}")
              for ch in range(NCH)]

        with tc.tile_pool(name="psh2", bufs=2, space="PSUM") as ps_h2, \
             tc.tile_pool(name="psph", bufs=4, space="PSUM") as ps_ph:
            for ch in range(NCH):
                bs = slice(ch * 128, (ch + 1) * 128)
                for w in range(WLEN):
                    hs_slice = HsTv[:, w, bs]
                    p2 = ps_h2.tile([128, 128], BF16, tag="h2")
                    nc.tensor.transpose(p2, hs_slice, identity_bf)
                    nc.vector.tensor_copy(Hs2[ch][:, :, w], p2)
                    pp = ps_ph.tile([128, H], F32, tag="ph")
                    nc.tensor.matmul(pp, lhsT=hs_slice, rhs=taW1hT,
                                     start=True, stop=False)
                    nc.tensor.matmul(pp, lhsT=ones1, rhs=tab1row,
                                     start=False, stop=True)
                    nc.vector.tensor_copy(PH[ch][:, w, :], pp)

        hsT_stack.close()

        # ------------------------------------------------------------------
        # decoder loop
        # ------------------------------------------------------------------
        dec = ExitStack()
        p_vp = dec.enter_context(tc.tile_pool(name="vp", bufs=2))
        p_v = dec.enter_context(tc.tile_pool(name="v", bufs=2))
        p_tr2 = dec.enter_context(tc.tile_pool(name="tr2", bufs=2))
        p_l = dec.enter_context(tc.tile_pool(name="l", bufs=4))
        p_s2 = dec.enter_context(tc.tile_pool(name="s2", bufs=4))
        p_bb = dec.enter_context(tc.tile_pool(name="bb", bufs=2))
        p_pdb = dec.enter_context(tc.tile_pool(name="pdb", bufs=2))
        p_ct = dec.enter_context(tc.tile_pool(name="ct", bufs=2))
        p_ctT = dec.enter_context(tc.tile_pool(name="ctT", bufs=2))
        p_yt = dec.enter_context(tc.tile_pool(name="yt", bufs=2))
        p_osb = dec.enter_context(tc.tile_pool(name="osb", bufs=2))
        p_out = dec.enter_context(tc.tile_pool(name="outT", bufs=2))
        p_th2 = dec.enter_context(tc.tile_pool(name="th2", bufs=6))
        p_tmp2 = dec.enter_context(tc.tile_pool(name="tmp2", bufs=3))
        p_dst = dec.enter_context(tc.tile_pool(name="dst", bufs=2))
        ps_pd = dec.enter_context(tc.tile_pool(name="pspd", bufs=2, space="PSUM"))
        ps_g2 = dec.enter_context(tc.tile_pool(name="psg2", bufs=3, space="PSUM"))
        ps_c = dec.enter_context(tc.tile_pool(name="psc", bufs=1, space="PSUM"))
        ps_mm = dec.enter_context(tc.tile_pool(name="psmm", bufs=1, space="PSUM"))
        ps_o = dec.enter_context(tc.tile_pool(name="pso", bufs=1, space="PSUM"))

        dT = pst.tile([H, BL], F32, tag="d0")
        dsT = pst.tile([H, BL], F32, tag="ds0")
        outT = pst.tile([1, BL], F32, tag="out0")
        nc.vector.memset(dT, 0.0)
        nc.vector.memset(dsT, 0.0)
        nc.vector.memset(outT, 0.0)

        # tree tags shared by the two per-chunk reduces (same byte sizes)
        def tree_reduce(r, width, tag_prefix):
            sizes = []
            sz = width // 2
            while sz >= 2:
                sizes.append(sz)
                sz //= 2
            mid = r.shape[1]
            for sz in sizes:
                nxt = p_tr2.tile([128, mid, sz], BF16,
                                 tag=f"{tag_prefix}{mid * sz * 2}")
                nc.vector.tensor_tensor(nxt, r[:, :, :sz], r[:, :, sz:2 * sz],
                                        op=ALU.add)
                r = nxt
            return r

        for t in range(WLEN):
            ctT = p_ctT.tile([H, BL], F32, tag="ctT")
            for ch in range(NCH):
                bs = slice(ch * 128, (ch + 1) * 128)
                # pd[b, n] = d taW1d^T + ds taW1s^T
                pps = ps_pd.tile([128, H], F32, tag="pd")
                nc.tensor.matmul(pps, lhsT=dT[:, bs], rhs=taW1dT, start=True, stop=False)
                nc.tensor.matmul(pps, lhsT=dsT[:, bs], rhs=taW1sT, start=False, stop=True)
                pdb = p_pdb.tile([128, H], BF16, tag="pdb")
                nc.vector.tensor_copy(pdb, pps)
                # v = tanh(PH + pd)
                vp = p_vp.tile([128, WLEN, H], BF16, tag="vp")
                nc.vector.tensor_tensor(vp, PH[ch], _bcast(pdb, WLEN), op=ALU.add)
                vv = p_v.tile([128, WLEN, H], BF16, tag="v")
                nc.scalar.activation(vv, vp, AF.Tanh)
                # l[b, w] = sum_n taW2[n] v[b, w, n]
                wv = p_vp.tile([128, WLEN, H], BF16, tag="vp")
                nc.vector.tensor_tensor(wv, vv, _bcast(taW2rep, WLEN), op=ALU.mult)
                r = tree_reduce(wv, H, "t")
                l = p_l.tile([128, WLEN], F32, tag="l")
                nc.vector.tensor_tensor(l, r[:, :, 0], r[:, :, 1], op=ALU.add)
                # softmax over w
                exl = p_l.tile([128, WLEN], F32, tag="l")
                nc.scalar.activation(exl, l, AF.Exp)
                S = p_s2.tile([128, 1], F32, tag="s2")
                nc.vector.reduce_sum(S, exl, axis=AX.X)
                Sr = p_s2.tile([128, 1], F32, tag="s2")
                nc.vector.reciprocal(Sr, S)
                beta = p_l.tile([128, WLEN], F32, tag="l")
                nc.vector.tensor_scalar_mul(beta, exl, Sr)
                betab = p_bb.tile([128, WLEN], BF16, tag="bb")
                nc.vector.tensor_copy(betab, beta)
                # ct[b, n] = sum_w beta[b, w] Hs2[b, n, w]
                pm = p_vp.tile([128, H, WLEN], BF16, tag="vp")
                nc.vector.tensor_tensor(pm, Hs2[ch], _bcast(betab, H), op=ALU.mult)
                r = tree_reduce(pm, WLEN, "t")
                ct = p_ct.tile([128, H], F32, tag="ct")
                nc.vector.tensor_tensor(ct, r[:, :, 0], r[:, :, 1], op=ALU.add)
                cps = ps_c.tile([128, 128], F32, tag="cT")
                nc.tensor.transpose(cps, ct, identity)
                nc.vector.tensor_copy(ctT[:, bs], cps)

            # yt^T = l1_W[:,1:] ct^T + l1_W[:,0] out^T + l1_b
            yps = ps_mm.tile([1, BL], F32, tag="mm")
            nc.tensor.matmul(yps, lhsT=l1wct, rhs=ctT, start=True, stop=False)
            nc.tensor.matmul(yps, lhsT=l1w0, rhs=outT, start=False, stop=True)
            ytT = p_yt.tile([1, BL], F32, tag="ytT")
            nc.scalar.activation(ytT, yps, AF.Identity, bias=l1brow)

            # decoder LSTM gates
            th = {}
            for i, nm in enumerate("ifgo"):
                gp = ps_g2.tile([H, BL], F32, tag="g2")
                nc.tensor.matmul(gp, lhsT=decWihR[:, i * H:(i + 1) * H], rhs=ytT,
                                 start=True, stop=False)
                nc.tensor.matmul(gp, lhsT=decWhhT[:, i * H:(i + 1) * H], rhs=dT,
                                 start=False, stop=True)
                tht = p_th2.tile([H, BL], F32, tag="th2")
                if nm == "g":
                    nc.scalar.activation(tht, gp, AF.Tanh, bias=bdec[:, 2:3])
                else:
                    nc.scalar.activation(tht, gp, AF.Tanh, bias=bdech[:, i:i + 1],
                                         scale=0.5)
                th[nm] = tht
            dN = p_dst.tile([H, BL], F32, tag="d")
            dsN = p_dst.tile([H, BL], F32, tag="ds")
            t1 = p_tmp2.tile([H, BL], F32, tag="tmp2")
            amr(t1, th["f"], dsT)
            t2 = p_tmp2.tile([H, BL], F32, tag="tmp2")
            amr(t2, th["i"], th["g"])
            nc.vector.tensor_add(dsN, t1, t2)
            thc = p_th2.tile([H, BL], F32, tag="th2")
            nc.scalar.activation(thc, dsN, AF.Tanh)
            amr(dN, th["o"], thc)

            # o^T = l2ct ct^T + l2d d^T + l2b ;  out = sigmoid(l3 o^T + l3b)
            ops_ = ps_o.tile([H, BL], F32, tag="o")
            nc.tensor.matmul(ops_, lhsT=l2WctT, rhs=ctT, start=True, stop=False)
            nc.tensor.matmul(ops_, lhsT=l2WdT, rhs=dN, start=False, stop=True)
            osb = p_osb.tile([H, BL], F32, tag="osb")
            nc.scalar.activation(osb, ops_, AF.Identity, bias=l2bcol)
            ups = ps_mm.tile([1, BL], F32, tag="mm")
            nc.tensor.matmul(ups, lhsT=l3wT, rhs=osb, start=True, stop=True)
            tho = p_yt.tile([1, BL], F32, tag="tho")
            nc.scalar.activation(tho, ups, AF.Tanh, bias=l3bh, scale=0.5)
            oN = p_out.tile([1, BL], F32, tag="outT")
            nc.vector.tensor_scalar(oN, tho, 0.5, 0.5, op0=ALU.mult, op1=ALU.add)

            dT, dsT, outT = dN, dsN, oN

        nc.sync.dma_start(out_ap.rearrange("a b -> b a"), outT)
        dec.close()
        dec_stack.close()


_CACHE = {}


def _get_compiled():
    if "nc" in _CACHE:
        return _CACHE["nc"]
    nc = bacc.Bacc("TRN2", target_bir_lowering=False, debug=False,
                   num_devices=NCORES)
    ins = {}
    ins["X"] = nc.dram_tensor("X", [BL, WLEN, F], F32, kind="ExternalInput").ap()
    for name, shape in WEIGHT_SPECS.items():
        ins[name] = nc.dram_tensor(name, list(shape), F32,
                                   kind="ExternalInput").ap()
    out = nc.dram_tensor("out", [BL, 1], F32, kind="ExternalOutput")
    with tile.TileContext(nc) as tc:
        build_kernel(tc, out.ap(), ins)
    nc.compile()
    _CACHE["nc"] = nc
    return nc


def kernel(**inputs):
    nc = _get_compiled()
    X = np.ascontiguousarray(np.asarray(inputs["X"], dtype=np.float32))
    weights = {k: np.ascontiguousarray(np.asarray(inputs[k], dtype=np.float32))
               for k in WEIGHT_SPECS}
    in_maps = []
    for m in range(NCORES):
        im = {"X": X[m * BL:(m + 1) * BL]}
        im.update(weights)
        in_maps.append(im)
    res = run_bass_kernel_spmd(nc, in_maps, core_ids=list(range(NCORES)),
                               trace=bool(int(os.environ.get("DARNN_TRACE", "0"))))
    if res.exec_time_ns is not None:
        print(f"HW exec time: {res.exec_time_ns} ns", file=sys.stderr)
    _CACHE["last_result"] = res
    return np.concatenate([r["out"] for r in res.results], axis=0)


if __name__ == "__main__":
    nc = _get_compiled()
    print("compiled OK")
